# revision 12
# baseline (speedup 1.0000x reference)
"""Trainium2 Bass kernel for nn_CoreAttention (S=2048, B=1, H=16, D=128).

Sharding: 16 heads across 8 NeuronCores (2 heads/core, tensor parallel).

Per head (big tensors stay feature-major so nothing large is transposed
on device; the host supplies Q^T/K^T/V^T per head):
    qT     = (Wqk^T Q^T) / NF            (bf16 PE, fp32 PSUM)
    kT     = Wqk^T K^T                   (bf16 PE)
    scoresT[k,q] = kT-block^T @ qT       (bf16 PE; causal: only q >= k)
    scoresT += causal mask on diag block (PE accumulate of -1e4 tile)
    expT   = exp(scoresT)                (ACT, PSUM->SBUF bf16)
    sums[q]= ones-matmuls over expT      (PE, N=1 column sums)
    v      = V^T-chunks^T @ Wv           (bf16 PE -> natural [s,e] layout)
    ctxT   = sum_j v_j^T @ expT_j        (bf16 PE, fp32 accum)
    ctx    = transpose(ctxT) * (1/sums)  (fp32 PE transpose + DVE scale)

exp() runs without max-subtraction: scores are ~N(0,1) (the reference
normalizes by sqrt(128)), so exp never overflows and matches the
reference's masked softmax to rounding error.
"""

import sys
from contextlib import ExitStack

import numpy as np

for _p in ("/opt/trn_rl_repo",):
    if _p not in sys.path:
        sys.path.insert(0, _p)

import ml_dtypes
import concourse.bass as bass
import concourse.tile as tile
from concourse import bacc, mybir
from concourse.bass_utils import run_bass_kernel_spmd

S, B, H, D = 2048, 1, 16, 128
HPC = 2  # heads per core
NCORES = 8
NB = S // 128  # 16 seq blocks of 128
NF = float(np.sqrt(2048.0 / 16.0))  # NORM_FACTOR
NEG = -10000.0
PAD = 384  # zero-pad columns in front of each expt_j buffer

F32 = mybir.dt.float32
BF16 = mybir.dt.bfloat16
AF = mybir.ActivationFunctionType


def build_program() -> bass.Bass:
    nc = bacc.Bacc(
        "TRN2", target_bir_lowering=False, debug=False, num_devices=NCORES
    )

    qt_d = nc.dram_tensor("qt", [HPC, D, S], F32, kind="ExternalInput")
    kt_d = nc.dram_tensor("kt", [HPC, D, S], F32, kind="ExternalInput")
    vt_d = nc.dram_tensor("vt", [HPC, D, S], F32, kind="ExternalInput")
    wqk_d = nc.dram_tensor("wqk", [HPC, D, D], F32, kind="ExternalInput")
    wv_d = nc.dram_tensor("wv", [HPC, D, D], F32, kind="ExternalInput")
    identf_d = nc.dram_tensor("identf", [D, D], F32, kind="ExternalInput")
    identb_d = nc.dram_tensor("identb", [D, D], BF16, kind="ExternalInput")
    maskb_d = nc.dram_tensor("maskb", [D, D], BF16, kind="ExternalInput")
    onesb_d = nc.dram_tensor("onesb", [D, 1], BF16, kind="ExternalInput")
    onesf_d = nc.dram_tensor("onesf", [1, 1], F32, kind="ExternalInput")
    out_d = nc.dram_tensor("out", [HPC, S, D], F32, kind="ExternalOutput")

    with tile.TileContext(nc) as tc, ExitStack() as ctx:
        cpool = ctx.enter_context(tc.tile_pool(name="const", bufs=1))
        sb = ctx.enter_context(tc.tile_pool(name="sb", bufs=1))
        ps = ctx.enter_context(tc.tile_pool(name="ps", bufs=1, space="PSUM"))

        identf = cpool.tile([D, D], F32)
        nc.sync.dma_start(identf[:], identf_d[:])
        identb = cpool.tile([D, D], BF16)
        nc.sync.dma_start(identb[:], identb_d[:])
        maskb = cpool.tile([D, D], BF16)
        nc.sync.dma_start(maskb[:], maskb_d[:])
        onesb = cpool.tile([D, 1], BF16)
        nc.sync.dma_start(onesb[:], onesb_d[:])
        onesf = cpool.tile([1, 1], F32)
        nc.sync.dma_start(onesf[:], onesf_d[:])

        # Warm the PE's view of identf's DMA queue so later fp32 transposes
        # (self-loading, max 1 sync wait) never need a second wait.
        warm_ps = ps.tile([D, D], F32, tag="otr", name="warm_ps")
        nc.tensor.transpose(warm_ps[:], identf[:], identf[:])

        for h in range(HPC):
            # ---- load raw inputs ------------------------------------------
            qtr = sb.tile([D, S], F32, tag="qtr", bufs=2)
            nc.sync.dma_start(qtr[:], qt_d[h])
            ktr = sb.tile([D, S], F32, tag="ktr", bufs=2)
            nc.sync.dma_start(ktr[:], kt_d[h])
            vtr = sb.tile([D, S], F32, tag="vtr", bufs=2)
            nc.sync.dma_start(vtr[:], vt_d[h])
            wqk = sb.tile([D, D], F32, tag="wqk", bufs=2)
            nc.sync.dma_start(wqk[:], wqk_d[h])
            wv = sb.tile([D, D], F32, tag="wv", bufs=2)
            nc.sync.dma_start(wv[:], wv_d[h])

            # ---- bf16 casts (DVE, 2 elem/cycle SBUF->SBUF) ----------------
            qtb = sb.tile([D, S], BF16, tag="qtb", bufs=1)
            nc.vector.tensor_copy(qtb[:], qtr[:])
            ktb = sb.tile([D, S], BF16, tag="ktb", bufs=1)
            nc.vector.tensor_copy(ktb[:], ktr[:])
            vtb = sb.tile([D, S], BF16, tag="vtb", bufs=1)
            nc.vector.tensor_copy(vtb[:], vtr[:])
            wqkb = sb.tile([D, D], BF16, tag="wqkb", bufs=2)
            nc.vector.tensor_copy(wqkb[:], wqk[:])
            wvb = sb.tile([D, D], BF16, tag="wvb", bufs=2)
            nc.vector.tensor_copy(wvb[:], wv[:])

            # ---- projections: qT = Wqk^T Q^T / NF,  kT = Wqk^T K^T --------
            qmt = sb.tile([D, S], BF16, tag="qmt", bufs=2)
            kmt = sb.tile([D, S], BF16, tag="kmt", bufs=2)
            for src, dst, scale in ((qtb, qmt, 1.0 / NF), (ktb, kmt, 1.0)):
                for c in range(2):
                    p = ps.tile(
                        [D, S // 2], F32, tag="big", bufs=2,
                        name=f"proj_ps_{h}_{dst.tensor.name}_{c}",
                    )
                    for c2 in range(2):
                        nc.tensor.matmul(
                            p[:, c2 * 512 : (c2 + 1) * 512],
                            wqkb[:],
                            src[:, c * 1024 + c2 * 512 : c * 1024 + (c2 + 1) * 512],
                        )
                    nc.scalar.activation(
                        dst[:, c * 1024 : (c + 1) * 1024], p[:], AF.Copy, scale=scale
                    )

            # ---- v chunks in natural [s,e] layout: v = V_raw @ Wv ---------
            vsb = sb.tile([D, NB * D], BF16, tag="vsb", bufs=2)
            for c in range(2):
                vp = ps.tile([D, S // 2], F32, tag="big", bufs=2, name=f"vp_ps_{h}_{c}")
                for j in range(8):
                    nc.tensor.matmul(
                        vp[:, j * 128 : (j + 1) * 128],
                        vtb[:, (c * 8 + j) * 128 : (c * 8 + j + 1) * 128],
                        wvb[:],
                    )
                nc.vector.tensor_copy(vsb[:, c * 1024 : (c + 1) * 1024], vp[:])

            # ---- pass 1: scoresT -> exp(bf16), left-padded with zeros -----
            # expt_j buffer holds PAD zero columns then the w real columns,
            # so later N=512 reads spanning "before the diagonal" see zeros.
            expts = []
            for j in range(NB):
                w = S - j * 128  # sq columns j*128 .. S
                expt = sb.tile(
                    [D, PAD + w], BF16, tag=f"expt{j}", bufs=2, name=f"expt_h{h}_{j}"
                )
                nc.gpsimd.memset(expt[:, 0:PAD], 0.0)
                nhalf = (w + 1023) // 1024
                for c in range(nhalf):
                    lo = c * 1024
                    cw = min(1024, w - lo)
                    sc_ps = ps.tile(
                        [D, cw], F32, tag="big", bufs=2, name=f"sc_ps_h{h}_{j}_{c}"
                    )
                    for c2 in range(0, cw, 512):
                        ce = min(c2 + 512, cw)
                        first = c == 0 and c2 == 0
                        nc.tensor.matmul(
                            sc_ps[:, c2:ce],
                            kmt[:, j * 128 : (j + 1) * 128],
                            qmt[:, j * 128 + lo + c2 : j * 128 + lo + ce],
                            start=True,
                            stop=not first,
                        )
                        if first:
                            # causal mask on diagonal block via PE accumulate
                            nc.tensor.matmul(
                                sc_ps[:, 0:128],
                                identb[:],
                                maskb[:],
                                start=False,
                                stop=True,
                            )
                    nc.scalar.activation(
                        expt[:, PAD + lo : PAD + lo + cw], sc_ps[:], AF.Exp
                    )
                expts.append(expt)

            # ---- softmax sums: ones-stationary N=512 row-sums -------------
            recip_ps = ps.tile([D, NB], F32, tag="recipps", name=f"recip_ps_{h}")
            for c in range(4):
                srow = ps.tile([1, 512], F32, tag="sumsrow", name=f"srow_{h}_{c}")
                njc = 4 * c + 4  # j = 0 .. 4c+3 contribute to this chunk
                for j in range(njc):
                    nc.tensor.matmul(
                        srow[:],
                        onesb[:],
                        expts[j][:, PAD + 512 * c - 128 * j : PAD + 512 * c - 128 * j + 512],
                        start=(j == 0),
                        stop=(j == njc - 1),
                    )
                srow_sb = sb.tile([1, 512], F32, tag="srow_sb", bufs=2)
                nc.vector.tensor_copy(srow_sb[:], srow[:])
                for s4 in range(4):
                    i = c * 4 + s4
                    # [1,128] row -> [128,1] column via K=1 matmul
                    nc.tensor.matmul(
                        recip_ps[:, i : i + 1],
                        srow_sb[0:1, s4 * 128 : (s4 + 1) * 128],
                        onesf[:],
                    )
            recip = sb.tile([D, NB], F32, tag="recip", bufs=2)
            nc.vector.reciprocal(recip[:], recip_ps[:])

            # ---- pass 2: PV accumulation, transpose, normalize, store -----
            for i4 in range(NB // 4):
                outt_ps = ps.tile([D, 512], F32, tag="outt", name=f"outt_{h}_{i4}")
                njc = 4 * i4 + 4
                for j in range(njc):
                    nc.tensor.matmul(
                        outt_ps[:],
                        vsb[:, j * 128 : (j + 1) * 128],
                        expts[j][:, PAD + 512 * i4 - 128 * j : PAD + 512 * i4 - 128 * j + 512],
                        start=(j == 0),
                        stop=(j == njc - 1),
                    )
                outt_sb = sb.tile([D, 512], F32, tag="outt_sb", bufs=2)
                nc.vector.tensor_copy(outt_sb[:], outt_ps[:])
                otr_ps = ps.tile([D, 512], F32, tag="otr", name=f"otr_{h}_{i4}")
                osb = sb.tile([D, 512], F32, tag="osb", bufs=2)
                for s4 in range(4):
                    i = i4 * 4 + s4
                    sl = slice(s4 * 128, (s4 + 1) * 128)
                    nc.tensor.transpose(otr_ps[:, sl], outt_sb[:, sl], identf[:])
                    nc.vector.tensor_scalar_mul(
                        osb[:, sl], otr_ps[:, sl], recip[:, i : i + 1]
                    )
                nc.sync.dma_start(
                    out_d[h, i4 * 512 : (i4 + 1) * 512, :].rearrange(
                        "(b s) e -> s b e", b=4
                    ),
                    osb[:].rearrange("p (b e) -> p b e", b=4),
                )

    nc.compile()
    return nc


_NC_CACHE = None


def _get_program():
    global _NC_CACHE
    if _NC_CACHE is None:
        _NC_CACHE = build_program()
    return _NC_CACHE


def make_in_maps(query_layer, key_layer, value_layer, svd_qk, svd_v):
    qt = np.ascontiguousarray(query_layer[:, 0].transpose(1, 2, 0))
    kt = np.ascontiguousarray(key_layer[:, 0].transpose(1, 2, 0))
    vt = np.ascontiguousarray(value_layer[:, 0].transpose(1, 2, 0))
    svd_qk = np.ascontiguousarray(svd_qk, dtype=np.float32)
    svd_v = np.ascontiguousarray(svd_v, dtype=np.float32)

    identf = np.eye(D, dtype=np.float32)
    identb = np.eye(D, dtype=ml_dtypes.bfloat16)
    r = np.arange(D)
    maskb = np.where(r[:, None] > r[None, :], NEG, 0.0).astype(ml_dtypes.bfloat16)
    onesb = np.ones((D, 1), dtype=ml_dtypes.bfloat16)

    in_maps = []
    for c in range(NCORES):
        hs = slice(c * HPC, (c + 1) * HPC)
        in_maps.append(
            {
                "qt": qt[hs],
                "kt": kt[hs],
                "vt": vt[hs],
                "wqk": svd_qk[hs],
                "wv": svd_v[hs],
                "identf": identf,
                "identb": identb,
                "maskb": maskb,
                "onesb": onesb,
                "onesf": np.ones((1, 1), dtype=np.float32),
            }
        )
    return in_maps


def assemble_output(results):
    out = np.empty((S, B, H * D), dtype=np.float32)
    for c in range(NCORES):
        o = results[c]["out"]  # [HPC, S, D]
        for hl in range(HPC):
            h = c * HPC + hl
            out[:, 0, h * D : (h + 1) * D] = o[hl]
    return out


def kernel(query_layer, key_layer, value_layer, attention_mask, svd_qk, svd_v):
    nc = _get_program()
    in_maps = make_in_maps(query_layer, key_layer, value_layer, svd_qk, svd_v)
    res = run_bass_kernel_spmd(nc, in_maps, list(range(NCORES))).results
    return assemble_output(res)


# revision 13
# speedup vs baseline: 1.0115x; 1.0115x over previous
"""Trainium2 Bass kernel for nn_CoreAttention (S=2048, B=1, H=16, D=128).

Sharding: 16 heads across 8 NeuronCores (2 heads/core, tensor parallel).

Per head (big tensors stay feature-major so nothing large is transposed
on device; the host supplies Q^T/K^T/V^T per head):
    qT     = (Wqk^T Q^T) / NF            (bf16 PE, fp32 PSUM)
    kT     = Wqk^T K^T                   (bf16 PE)
    scoresT[k,q] = kT-block^T @ qT       (bf16 PE; causal: only q >= k)
    scoresT += causal mask on diag block (PE accumulate of -1e4 tile)
    expT   = exp(scoresT)                (ACT, PSUM->SBUF bf16)
    sums[q]= ones-matmuls over expT      (PE, N=1 column sums)
    v      = V^T-chunks^T @ Wv           (bf16 PE -> natural [s,e] layout)
    ctxT   = sum_j v_j^T @ expT_j        (bf16 PE, fp32 accum)
    ctx    = transpose(ctxT) * (1/sums)  (fp32 PE transpose + DVE scale)

exp() runs without max-subtraction: scores are ~N(0,1) (the reference
normalizes by sqrt(128)), so exp never overflows and matches the
reference's masked softmax to rounding error.
"""

import sys
from contextlib import ExitStack

import numpy as np

for _p in ("/opt/trn_rl_repo",):
    if _p not in sys.path:
        sys.path.insert(0, _p)

import ml_dtypes
import concourse.bass as bass
import concourse.tile as tile
from concourse import bacc, mybir
from concourse.bass_utils import run_bass_kernel_spmd

S, B, H, D = 2048, 1, 16, 128
HPC = 2  # heads per core
NCORES = 8
NB = S // 128  # 16 seq blocks of 128
NF = float(np.sqrt(2048.0 / 16.0))  # NORM_FACTOR
NEG = -10000.0
PAD = 384  # zero-pad columns in front of each expt_j buffer

F32 = mybir.dt.float32
BF16 = mybir.dt.bfloat16
AF = mybir.ActivationFunctionType


def build_program() -> bass.Bass:
    nc = bacc.Bacc(
        "TRN2", target_bir_lowering=False, debug=False, num_devices=NCORES
    )

    qt_d = nc.dram_tensor("qt", [HPC, D, S], F32, kind="ExternalInput")
    kt_d = nc.dram_tensor("kt", [HPC, D, S], F32, kind="ExternalInput")
    vt_d = nc.dram_tensor("vt", [HPC, D, S], F32, kind="ExternalInput")
    wqk_d = nc.dram_tensor("wqk", [HPC, D, D], F32, kind="ExternalInput")
    wv_d = nc.dram_tensor("wv", [HPC, D, D], F32, kind="ExternalInput")
    identf_d = nc.dram_tensor("identf", [D, D], F32, kind="ExternalInput")
    identb_d = nc.dram_tensor("identb", [D, D], BF16, kind="ExternalInput")
    maskb_d = nc.dram_tensor("maskb", [D, D], BF16, kind="ExternalInput")
    onesb_d = nc.dram_tensor("onesb", [D, 1], BF16, kind="ExternalInput")
    onesf_d = nc.dram_tensor("onesf", [1, 1], F32, kind="ExternalInput")
    out_d = nc.dram_tensor("out", [HPC, S, D], F32, kind="ExternalOutput")

    with tile.TileContext(nc) as tc, ExitStack() as ctx:
        cpool = ctx.enter_context(tc.tile_pool(name="const", bufs=1))
        sb = ctx.enter_context(tc.tile_pool(name="sb", bufs=1))
        ps = ctx.enter_context(tc.tile_pool(name="ps", bufs=1, space="PSUM"))

        identf = cpool.tile([D, D], F32)
        nc.sync.dma_start(identf[:], identf_d[:])
        identb = cpool.tile([D, D], BF16)
        nc.sync.dma_start(identb[:], identb_d[:])
        maskb = cpool.tile([D, D], BF16)
        nc.sync.dma_start(maskb[:], maskb_d[:])
        onesb = cpool.tile([D, 1], BF16)
        nc.sync.dma_start(onesb[:], onesb_d[:])
        onesf = cpool.tile([1, 1], F32)
        nc.sync.dma_start(onesf[:], onesf_d[:])

        # Warm the PE's view of identf's DMA queue so later fp32 transposes
        # (self-loading, max 1 sync wait) never need a second wait.
        warm_ps = ps.tile([D, D], F32, tag="otr", name="warm_ps")
        nc.tensor.transpose(warm_ps[:], identf[:], identf[:])

        for h in range(HPC):
            # ---- load raw inputs (weights first: tiny, unblock projs) -----
            wqk = sb.tile([D, D], F32, tag="wqk", bufs=2)
            nc.sync.dma_start(wqk[:], wqk_d[h])
            wv = sb.tile([D, D], F32, tag="wv", bufs=2)
            nc.sync.dma_start(wv[:], wv_d[h])
            wqkb = sb.tile([D, D], BF16, tag="wqkb", bufs=2)
            nc.vector.tensor_copy(wqkb[:], wqk[:])
            wvb = sb.tile([D, D], BF16, tag="wvb", bufs=2)
            nc.vector.tensor_copy(wvb[:], wv[:])

            # q/k/v loads + bf16 casts, pipelined at 1024-col granularity
            qtr = sb.tile([D, S], F32, tag="qtr", bufs=2)
            ktr = sb.tile([D, S], F32, tag="ktr", bufs=2)
            vtr = sb.tile([D, S], F32, tag="vtr", bufs=2)
            qtb = sb.tile([D, S], BF16, tag="qtb", bufs=1)
            ktb = sb.tile([D, S], BF16, tag="ktb", bufs=1)
            vtb = sb.tile([D, S], BF16, tag="vtb", bufs=1)
            for raw, dr, cast in ((qtr, qt_d, qtb), (ktr, kt_d, ktb), (vtr, vt_d, vtb)):
                for c in range(2):
                    sl = slice(c * 1024, (c + 1) * 1024)
                    nc.sync.dma_start(raw[:, sl], dr[h][:, sl])
                    nc.vector.tensor_copy(cast[:, sl], raw[:, sl])

            # ---- projections: qT = Wqk^T Q^T / NF,  kT = Wqk^T K^T --------
            qmt = sb.tile([D, S], BF16, tag="qmt", bufs=2)
            kmt = sb.tile([D, S], BF16, tag="kmt", bufs=2)
            for src, dst, scale in ((qtb, qmt, 1.0 / NF), (ktb, kmt, 1.0)):
                for c in range(2):
                    p = ps.tile(
                        [D, S // 2], F32, tag="big", bufs=2,
                        name=f"proj_ps_{h}_{dst.tensor.name}_{c}",
                    )
                    for c2 in range(2):
                        nc.tensor.matmul(
                            p[:, c2 * 512 : (c2 + 1) * 512],
                            wqkb[:],
                            src[:, c * 1024 + c2 * 512 : c * 1024 + (c2 + 1) * 512],
                        )
                    nc.scalar.activation(
                        dst[:, c * 1024 : (c + 1) * 1024], p[:], AF.Copy, scale=scale
                    )

            # ---- v chunks in natural [s,e] layout: v = V_raw @ Wv ---------
            vsb = sb.tile([D, NB * D], BF16, tag="vsb", bufs=2)
            for c in range(2):
                vp = ps.tile([D, S // 2], F32, tag="big", bufs=2, name=f"vp_ps_{h}_{c}")
                for j in range(8):
                    nc.tensor.matmul(
                        vp[:, j * 128 : (j + 1) * 128],
                        vtb[:, (c * 8 + j) * 128 : (c * 8 + j + 1) * 128],
                        wvb[:],
                    )
                nc.vector.tensor_copy(vsb[:, c * 1024 : (c + 1) * 1024], vp[:])

            # ---- pass 1: scoresT -> exp(bf16), left-padded with zeros -----
            # expt_j buffer holds PAD zero columns then the w real columns,
            # so later N=512 reads spanning "before the diagonal" see zeros.
            expts = []
            for j in range(NB):
                w = S - j * 128  # sq columns j*128 .. S
                expt = sb.tile(
                    [D, PAD + w], BF16, tag=f"expt{j}", bufs=2, name=f"expt_h{h}_{j}"
                )
                nc.gpsimd.memset(expt[:, 0:PAD], 0.0)
                nhalf = (w + 1023) // 1024
                for c in range(nhalf):
                    lo = c * 1024
                    cw = min(1024, w - lo)
                    sc_ps = ps.tile(
                        [D, cw], F32, tag="big", bufs=2, name=f"sc_ps_h{h}_{j}_{c}"
                    )
                    for c2 in range(0, cw, 512):
                        ce = min(c2 + 512, cw)
                        first = c == 0 and c2 == 0
                        nc.tensor.matmul(
                            sc_ps[:, c2:ce],
                            kmt[:, j * 128 : (j + 1) * 128],
                            qmt[:, j * 128 + lo + c2 : j * 128 + lo + ce],
                            start=True,
                            stop=not first,
                        )
                        if first:
                            # causal mask on diagonal block via PE accumulate
                            nc.tensor.matmul(
                                sc_ps[:, 0:128],
                                identb[:],
                                maskb[:],
                                start=False,
                                stop=True,
                            )
                    nc.scalar.activation(
                        expt[:, PAD + lo : PAD + lo + cw], sc_ps[:], AF.Exp
                    )
                expts.append(expt)

            # ---- softmax sums: ones-stationary N=512 row-sums -------------
            recip_ps = ps.tile([D, NB], F32, tag="recipps", name=f"recip_ps_{h}")
            for c in range(4):
                srow = ps.tile([1, 512], F32, tag="sumsrow", name=f"srow_{h}_{c}")
                njc = 4 * c + 4  # j = 0 .. 4c+3 contribute to this chunk
                for j in range(njc):
                    nc.tensor.matmul(
                        srow[:],
                        onesb[:],
                        expts[j][:, PAD + 512 * c - 128 * j : PAD + 512 * c - 128 * j + 512],
                        start=(j == 0),
                        stop=(j == njc - 1),
                    )
                srow_sb = sb.tile([1, 512], F32, tag="srow_sb", bufs=2)
                nc.vector.tensor_copy(srow_sb[:], srow[:])
                for s4 in range(4):
                    i = c * 4 + s4
                    # [1,128] row -> [128,1] column via K=1 matmul
                    nc.tensor.matmul(
                        recip_ps[:, i : i + 1],
                        srow_sb[0:1, s4 * 128 : (s4 + 1) * 128],
                        onesf[:],
                    )
            recip = sb.tile([D, NB], F32, tag="recip", bufs=2)
            nc.vector.reciprocal(recip[:], recip_ps[:])

            # ---- pass 2: PV accumulation, transpose, normalize, store -----
            for i4 in range(NB // 4):
                outt_ps = ps.tile([D, 512], F32, tag="outt", name=f"outt_{h}_{i4}")
                njc = 4 * i4 + 4
                for j in range(njc):
                    nc.tensor.matmul(
                        outt_ps[:],
                        vsb[:, j * 128 : (j + 1) * 128],
                        expts[j][:, PAD + 512 * i4 - 128 * j : PAD + 512 * i4 - 128 * j + 512],
                        start=(j == 0),
                        stop=(j == njc - 1),
                    )
                outt_sb = sb.tile([D, 512], F32, tag="outt_sb", bufs=2)
                nc.vector.tensor_copy(outt_sb[:], outt_ps[:])
                otr_ps = ps.tile([D, 512], F32, tag="otr", name=f"otr_{h}_{i4}")
                osb = sb.tile([D, 512], F32, tag="osb", bufs=2)
                for s4 in range(4):
                    i = i4 * 4 + s4
                    sl = slice(s4 * 128, (s4 + 1) * 128)
                    nc.tensor.transpose(otr_ps[:, sl], outt_sb[:, sl], identf[:])
                    nc.vector.tensor_scalar_mul(
                        osb[:, sl], otr_ps[:, sl], recip[:, i : i + 1]
                    )
                nc.sync.dma_start(
                    out_d[h, i4 * 512 : (i4 + 1) * 512, :].rearrange(
                        "(b s) e -> s b e", b=4
                    ),
                    osb[:].rearrange("p (b e) -> p b e", b=4),
                )

    nc.compile()
    return nc


_NC_CACHE = None


def _get_program():
    global _NC_CACHE
    if _NC_CACHE is None:
        _NC_CACHE = build_program()
    return _NC_CACHE


def make_in_maps(query_layer, key_layer, value_layer, svd_qk, svd_v):
    qt = np.ascontiguousarray(query_layer[:, 0].transpose(1, 2, 0))
    kt = np.ascontiguousarray(key_layer[:, 0].transpose(1, 2, 0))
    vt = np.ascontiguousarray(value_layer[:, 0].transpose(1, 2, 0))
    svd_qk = np.ascontiguousarray(svd_qk, dtype=np.float32)
    svd_v = np.ascontiguousarray(svd_v, dtype=np.float32)

    identf = np.eye(D, dtype=np.float32)
    identb = np.eye(D, dtype=ml_dtypes.bfloat16)
    r = np.arange(D)
    maskb = np.where(r[:, None] > r[None, :], NEG, 0.0).astype(ml_dtypes.bfloat16)
    onesb = np.ones((D, 1), dtype=ml_dtypes.bfloat16)

    in_maps = []
    for c in range(NCORES):
        hs = slice(c * HPC, (c + 1) * HPC)
        in_maps.append(
            {
                "qt": qt[hs],
                "kt": kt[hs],
                "vt": vt[hs],
                "wqk": svd_qk[hs],
                "wv": svd_v[hs],
                "identf": identf,
                "identb": identb,
                "maskb": maskb,
                "onesb": onesb,
                "onesf": np.ones((1, 1), dtype=np.float32),
            }
        )
    return in_maps


def assemble_output(results):
    out = np.empty((S, B, H * D), dtype=np.float32)
    for c in range(NCORES):
        o = results[c]["out"]  # [HPC, S, D]
        for hl in range(HPC):
            h = c * HPC + hl
            out[:, 0, h * D : (h + 1) * D] = o[hl]
    return out


def kernel(query_layer, key_layer, value_layer, attention_mask, svd_qk, svd_v):
    nc = _get_program()
    in_maps = make_in_maps(query_layer, key_layer, value_layer, svd_qk, svd_v)
    res = run_bass_kernel_spmd(nc, in_maps, list(range(NCORES))).results
    return assemble_output(res)
